# revision 1
# baseline (speedup 1.0000x reference)
"""Trainium2 Bass kernel for nn_Decoders (tri-plane MoE-routing decoder).

Takes FULL unsharded inputs, shards points data-parallel across 8 NeuronCores,
replicates the plane tables + MLP weights, and runs an SPMD Bass program:
  route points to submaps -> bilinear-gather 3 fused tri-plane tables
  -> two tiny MLPs -> [rgb, sdf] output.
"""

import os
import sys
import time

import numpy as np

import concourse.bass as bass
import concourse.bacc as bacc
import concourse.tile as tile
from concourse import mybir
from concourse.bass import IndirectOffsetOnAxis
from concourse.bass_utils import run_bass_kernel_spmd
from concourse.masks import make_identity

S, R, C, H = 8, 256, 32, 32
NCORES = 8
KJ = 16                  # points per partition per tile
PTILE = 128 * KJ         # 2048 points per tile
NT_FULL = 62             # tiles per core for the 1M-point problem
NTOT = 1000000

F32 = mybir.dt.float32
I32 = mybir.dt.int32
Alu = mybir.AluOpType
Act = mybir.ActivationFunctionType
AxX = mybir.AxisListType.X

NCELL = S * R * R        # 524288 cells per fused table; each cell = 64 f32

# consts layout (one flat f32 vector, broadcast to all partitions)
OFF_LO = 0      # [8,3] boundaries lo
OFF_HI = 24     # [8,3] boundaries hi
OFF_LOHI = 48   # [8,2,3] boundaries
OFF_W8 = 96     # [8] first-match weights 8-s
OFF_M3 = 104    # [3,5] index coefficient matrix (submap-local patch index)
OFF_BMIN = 119  # [3] per-core submap bmin
OFF_R255 = 122  # [3] per-core 255/(bmax-bmin)
NCONST = 125


def _v(t, off, dims):
    """Build a raw strided AP view on a tile/dram AP's tensor."""
    return bass.AP(t.tensor, off, [[s, c] for (s, c) in dims])


def _build_program(nt, dbg=False):
    """Build + compile the SPMD single-core program processing nt*2048 points."""
    nc = bacc.Bacc("TRN2", target_bir_lowering=False, debug=False,
                   enable_asserts=True)
    dbg_specs = [
        ("d_vec5", [128, KJ * 5], F32), ("d_wf", [128, 48], F32),
        ("d_valid", [128, KJ], F32), ("d_iall", [128, 96], I32),
        ("d_w12", [128, KJ * 12], F32), ("d_g0", [128, 2 * KJ * 128], F32),
        ("d_ff", [128, KJ * 64], F32), ("d_featT", [64, PTILE], F32),
        ("d_h1", [64, PTILE], F32), ("d_bb", [128, 96], F32),
    ]
    dbg_t = {}
    if dbg:
        for nm, shp, dt in dbg_specs:
            dbg_t[nm] = nc.dram_tensor(nm, shp, dt, kind="ExternalOutput")

    p_in = nc.dram_tensor("p_in", [nt, 128, KJ, 3], F32, kind="ExternalInput")
    v_in = nc.dram_tensor("v_in", [nt, 128, KJ], F32, kind="ExternalInput")
    # per-core patch tables: one submap, patch[y*256+x] = the 4 bilerp corner
    # cells (2x2) of the fused (planes|c_planes) table = 256 f32 = 1KB
    tabs = [nc.dram_tensor(f"tab{o}", [R * R, 256], F32, kind="ExternalInput")
            for o in range(3)]
    w1d = nc.dram_tensor("w1blk", [64, 64], F32, kind="ExternalInput")
    w2d = nc.dram_tensor("w2blk", [64, 64], F32, kind="ExternalInput")
    w3d = nc.dram_tensor("w3blk", [64, 36], F32, kind="ExternalInput")
    b1d = nc.dram_tensor("b1v", [64], F32, kind="ExternalInput")
    b2d = nc.dram_tensor("b2v", [64], F32, kind="ExternalInput")
    b3d = nc.dram_tensor("b3v", [4], F32, kind="ExternalInput")
    cstd = nc.dram_tensor("cst", [NCONST], F32, kind="ExternalInput")
    out4 = nc.dram_tensor("out4", [nt, 4, PTILE], F32, kind="ExternalOutput")

    with tile.TileContext(nc) as tc:
        with tc.tile_pool(name="const", bufs=1) as cp:
            # persistent constants
            ident = cp.tile([128, 128], F32)
            make_identity(nc, ident)
            ones1 = cp.tile([1, 128], F32)
            nc.vector.memset(ones1, 1.0)
            csb = cp.tile([1, NCONST], F32)
            nc.sync.dma_start(out=csb, in_=_v(cstd.ap(), 0, [(NCONST, 1), (1, NCONST)]))
            CB = cp.tile([128, NCONST], F32)
            with tc.tile_pool(name="setup_ps", bufs=1, space="PSUM") as sps:
                cb_ps = sps.tile([128, NCONST], F32)
                nc.tensor.matmul(out=cb_ps[:], lhsT=ones1[:], rhs=csb[:],
                                 start=True, stop=True)
                nc.scalar.copy(out=CB[:], in_=cb_ps[:])
            W1 = cp.tile([64, 64], F32)
            nc.sync.dma_start(out=W1, in_=w1d.ap())
            W2 = cp.tile([64, 64], F32)
            nc.sync.dma_start(out=W2, in_=w2d.ap())
            W3 = cp.tile([64, 36], F32)
            nc.sync.dma_start(out=W3, in_=w3d.ap())
            B1 = cp.tile([64, 1], F32)
            nc.sync.dma_start(out=B1, in_=_v(b1d.ap(), 0, [(1, 64), (1, 1)]))
            B2 = cp.tile([64, 1], F32)
            nc.sync.dma_start(out=B2, in_=_v(b2d.ap(), 0, [(1, 64), (1, 1)]))
            B3 = cp.tile([36, 1], F32)
            nc.sync.dma_start(out=B3[0:3, :], in_=_v(b3d.ap(), 0, [(1, 3), (1, 1)]))
            nc.sync.dma_start(out=B3[32:33, :], in_=_v(b3d.ap(), 3, [(1, 1), (1, 1)]))
            # all points, laid out [128part, (tile, j, c)]
            PA = cp.tile([128, nt * KJ * 3], F32)
            nc.sync.dma_start(
                out=_v(PA, 0, [(nt * 48, 128), (48, nt), (1, 48)]),
                in_=_v(p_in.ap(), 0, [(48, 128), (128 * 48, nt), (1, 48)]))
            VA = cp.tile([128, nt * KJ], F32)
            nc.sync.dma_start(
                out=_v(VA, 0, [(nt * KJ, 128), (KJ, nt), (1, KJ)]),
                in_=_v(v_in.ap(), 0, [(KJ, 128), (128 * KJ, nt), (1, KJ)]))

            with (
                tc.tile_pool(name="wrk", bufs=2) as wp,
                tc.tile_pool(name="gath", bufs=4) as gp,
                tc.tile_pool(name="big", bufs=1) as bp,
                tc.tile_pool(name="mlp", bufs=2) as mp,
                tc.tile_pool(name="ps", bufs=2, space="PSUM") as ps,
            ):
                for t in range(nt):
                    _tile_body(nc, tc, t, PA, VA, CB, ident, W1, W2, W3, B1, B2,
                               B3, tabs, out4, nt, wp, gp, bp, mp, ps,
                               dbg_t if (dbg and t == 0) else None)

    nc.compile()
    return nc


def _tile_body(nc, tc, t, PA, VA, CB, ident, W1, W2, W3, B1, B2, B3, tabs,
               out4, nt, wp, gp, bp, mp, ps, dbg_t=None):
    def ddump(name, ap):
        if dbg_t is not None and name in dbg_t:
            nc.sync.dma_start(out=dbg_t[name].ap(), in_=ap)
    PS = nt * KJ * 3  # partition stride of PA
    p3 = _v(PA, t * 48, [(PS, 128), (3, KJ), (1, 3)])          # [128, j, c]

    # ------- routing precomputed on host: this core = one submap -------
    valid = _v(VA, t * KJ, [(nt * KJ, 128), (1, KJ)])          # [128, j]
    vec5 = wp.tile([128, KJ * 5], F32)   # (j, [s(unused), gx, gy, gz, 1])
    nc.vector.memset(_v(vec5, 0, [(KJ * 5, 128), (5, KJ), (4, 2)]), 1.0)
    # g = (p - bmin) * (255/(bmax-bmin)) with per-core constants
    tnum = wp.tile([128, 48], F32)
    nc.vector.tensor_tensor(
        out=tnum[:], in0=p3,
        in1=_v(CB, OFF_BMIN, [(NCONST, 128), (0, KJ), (1, 3)]),
        op=Alu.subtract)
    g = wp.tile([128, 48], F32)
    nc.vector.tensor_tensor(
        out=g[:], in0=tnum[:],
        in1=_v(CB, OFF_R255, [(NCONST, 128), (0, KJ), (1, 3)]),
        op=Alu.mult)
    # floor(g) via round-to-nearest (add/sub 2^23) then fix-up where rnd > g
    grnd = wp.tile([128, 48], F32)
    nc.vector.tensor_scalar(out=grnd[:], in0=g[:], scalar1=8388608.0,
                            scalar2=-8388608.0, op0=Alu.add, op1=Alu.add)
    gfix = wp.tile([128, 48], F32)
    nc.vector.tensor_tensor(out=gfix[:], in0=grnd[:], in1=g[:], op=Alu.is_gt)
    g0 = wp.tile([128, 48], F32)
    nc.vector.tensor_tensor(out=g0[:], in0=grnd[:], in1=gfix[:], op=Alu.subtract)
    # clipped integer cell coords -> vec5[:, :, 1:4]
    nc.vector.tensor_scalar(
        out=_v(vec5, 1, [(KJ * 5, 128), (5, KJ), (1, 3)]),
        in0=g0[:], scalar1=0.0, scalar2=float(R - 2), op0=Alu.max, op1=Alu.min)
    wf = wp.tile([128, 48], F32)
    nc.vector.tensor_tensor(
        out=wf[:], in0=g[:],
        in1=_v(vec5, 1, [(KJ * 5, 128), (5, KJ), (1, 3)]),
        op=Alu.subtract)

    ddump("d_wf", wf[:])  # noqa
    ddump("d_valid", valid)
    ddump("d_vec5", vec5[:])

    # ---------------- gather patch indices (y*256 + x, submap-local) ------
    t4 = wp.tile([128, 240], F32)      # (j, o3, c5)
    nc.vector.tensor_tensor(
        out=_v(t4, 0, [(240, 128), (15, KJ), (5, 3), (1, 5)]),
        in0=_v(vec5, 0, [(KJ * 5, 128), (5, KJ), (0, 3), (1, 5)]),
        in1=_v(CB, OFF_M3, [(NCONST, 128), (0, KJ), (5, 3), (1, 5)]),
        op=Alu.mult)
    idxf = wp.tile([128, 48], F32)     # (j, o)
    nc.vector.tensor_reduce(
        out=idxf[:], in_=_v(t4, 0, [(240, 128), (5, 48), (1, 5)]),
        axis=AxX, op=Alu.add)
    iall = wp.tile([128, 48], I32)     # (o, j)
    nc.vector.tensor_copy(
        out=_v(iall, 0, [(48, 128), (16, 3), (1, KJ)]),
        in_=_v(idxf, 0, [(48, 128), (1, 3), (3, KJ)]))

    ddump("d_iall", iall[:])

    # ---------------- bilerp weights (valid-masked) ----------------
    a48 = wp.tile([128, 48], F32)      # 1 - wf
    nc.vector.tensor_scalar(out=a48[:], in0=wf[:], scalar1=-1.0, scalar2=1.0,
                            op0=Alu.mult, op1=Alu.add)
    yw = wp.tile([128, 96], F32)       # (j, o3, yb2)
    # yb=0: (1-wv)*valid  with vcol = [y, z, z]
    nc.vector.tensor_tensor(
        out=_v(yw, 0, [(96, 128), (6, KJ)]),
        in0=_v(a48, 1, [(48, 128), (3, KJ)]),
        in1=_v(VA, t * KJ, [(nt * KJ, 128), (1, KJ)]), op=Alu.mult)
    nc.vector.tensor_tensor(
        out=_v(yw, 2, [(96, 128), (6, KJ), (2, 2)]),
        in0=_v(a48, 2, [(48, 128), (3, KJ), (0, 2)]),
        in1=_v(VA, t * KJ, [(nt * KJ, 128), (1, KJ), (0, 2)]), op=Alu.mult)
    # yb=1: wv*valid
    nc.vector.tensor_tensor(
        out=_v(yw, 1, [(96, 128), (6, KJ)]),
        in0=_v(wf, 1, [(48, 128), (3, KJ)]),
        in1=_v(VA, t * KJ, [(nt * KJ, 128), (1, KJ)]), op=Alu.mult)
    nc.vector.tensor_tensor(
        out=_v(yw, 3, [(96, 128), (6, KJ), (2, 2)]),
        in0=_v(wf, 2, [(48, 128), (3, KJ), (0, 2)]),
        in1=_v(VA, t * KJ, [(nt * KJ, 128), (1, KJ), (0, 2)]), op=Alu.mult)
    w12 = bp.tile([128, KJ * 12], F32)  # (j, o, yb, xb)
    # xb=0: (1-wu)*yw with ucol = [x, x, y]
    nc.vector.tensor_tensor(
        out=_v(w12, 0, [(KJ * 12, 128), (12, KJ), (4, 2), (2, 2)]),
        in0=_v(a48, 0, [(48, 128), (3, KJ), (0, 2), (0, 2)]),
        in1=_v(yw, 0, [(96, 128), (6, KJ), (2, 2), (1, 2)]),
        op=Alu.mult)
    nc.vector.tensor_tensor(
        out=_v(w12, 8, [(KJ * 12, 128), (12, KJ), (2, 2)]),
        in0=_v(a48, 1, [(48, 128), (3, KJ), (0, 2)]),
        in1=_v(yw, 4, [(96, 128), (6, KJ), (1, 2)]),
        op=Alu.mult)
    # xb=1: wu*yw
    nc.vector.tensor_tensor(
        out=_v(w12, 1, [(KJ * 12, 128), (12, KJ), (4, 2), (2, 2)]),
        in0=_v(wf, 0, [(48, 128), (3, KJ), (0, 2), (0, 2)]),
        in1=_v(yw, 0, [(96, 128), (6, KJ), (2, 2), (1, 2)]),
        op=Alu.mult)
    nc.vector.tensor_tensor(
        out=_v(w12, 9, [(KJ * 12, 128), (12, KJ), (2, 2)]),
        in0=_v(wf, 1, [(48, 128), (3, KJ), (0, 2)]),
        in1=_v(yw, 4, [(96, 128), (6, KJ), (1, 2)]),
        op=Alu.mult)

    ddump("d_w12", w12[:])

    # ---------------- indirect gathers + weighted corner sums ------------
    ffs = []
    for o in range(3):
        g_t = gp.tile([128, KJ * 256], F32, name=f"g_t")
        # one index per partition per call (multi-index indirect DMA is
        # broken in the Q7 DGE; [128,1] is the verified-working form).
        # Each index fetches one 1KB patch = all 4 bilerp corner cells.
        for j in range(KJ):
            nc.gpsimd.indirect_dma_start(
                out=_v(g_t, j * 256, [(KJ * 256, 128), (1, 256)]),
                out_offset=None,
                in_=tabs[o].ap(),
                in_offset=IndirectOffsetOnAxis(
                    ap=_v(iall, o * KJ + j, [(48, 128), (1, 1)]), axis=0),
            )
        if o == 0:
            ddump("d_g0", g_t[:])
        p_o = bp.tile([128, KJ * 256], F32, name="p_o")  # (j, st, c, q4)
        nc.vector.tensor_tensor(
            out=_v(p_o, 0, [(KJ * 256, 128), (256, KJ), (4, 2 * C), (1, 4)]),
            in0=_v(g_t, 0, [(KJ * 256, 128), (256, KJ), (1, 2 * C), (64, 4)]),
            in1=_v(w12, o * 4, [(KJ * 12, 128), (12, KJ), (0, 2 * C), (1, 4)]),
            op=Alu.mult)
        ff_o = wp.tile([128, KJ * 64], F32, name="ff_o", bufs=3)  # (j, st, c)
        nc.vector.tensor_reduce(
            out=ff_o[:],
            in_=_v(p_o, 0, [(KJ * 256, 128), (4, KJ * 64), (1, 4)]),
            axis=AxX, op=Alu.add)
        ffs.append(ff_o)
    ff = ffs[0]
    nc.vector.tensor_tensor(out=ff[:], in0=ffs[0][:], in1=ffs[1][:], op=Alu.add)
    nc.vector.tensor_tensor(out=ff[:], in0=ff[:], in1=ffs[2][:], op=Alu.add)

    ddump("d_ff", ff[:])

    # ---------------- MLP ----------------
    featT_ps = ps.tile([64, PTILE], F32, tag="psbig", name="featT_ps")
    for j in range(KJ):
        nc.tensor.transpose(
            out=featT_ps[:, j * 128:(j + 1) * 128],
            in_=ff[:, j * 64:(j + 1) * 64],
            identity=ident[:])
    featT = mp.tile([64, PTILE], F32, bufs=1)
    nc.scalar.copy(out=featT[:], in_=featT_ps[:])
    h1ps = ps.tile([64, PTILE], F32, tag="psbig", name="h1ps")
    for ch in range(PTILE // 512):
        nc.tensor.matmul(out=h1ps[:, ch * 512:(ch + 1) * 512], lhsT=W1[:],
                         rhs=featT[:, ch * 512:(ch + 1) * 512],
                         start=True, stop=True)
    h1 = mp.tile([64, PTILE], F32, bufs=1)
    nc.scalar.activation(out=h1[:], in_=h1ps[:], func=Act.Relu, bias=B1[:],
                         scale=1.0)
    ddump("d_featT", featT[:])
    ddump("d_h1", h1[:])
    h2ps = ps.tile([64, PTILE], F32, tag="psbig", name="h2ps")
    for ch in range(PTILE // 512):
        nc.tensor.matmul(out=h2ps[:, ch * 512:(ch + 1) * 512], lhsT=W2[:],
                         rhs=h1[:, ch * 512:(ch + 1) * 512],
                         start=True, stop=True)
    h2 = mp.tile([64, PTILE], F32, bufs=1)
    nc.scalar.activation(out=h2[:], in_=h2ps[:], func=Act.Relu, bias=B2[:],
                         scale=1.0)
    o4ps = ps.tile([64, PTILE], F32, tag="psbig", name="o4ps")
    for ch in range(PTILE // 512):
        nc.tensor.matmul(out=o4ps[0:36, ch * 512:(ch + 1) * 512], lhsT=W3[:],
                         rhs=h2[:, ch * 512:(ch + 1) * 512],
                         start=True, stop=True)
    o4 = mp.tile([36, PTILE], F32)
    nc.scalar.activation(out=o4[0:3, :], in_=o4ps[0:3, :], func=Act.Sigmoid,
                         bias=B3[0:3, :], scale=1.0)
    nc.scalar.activation(out=o4[32:33, :], in_=o4ps[32:33, :], func=Act.Tanh,
                         bias=B3[32:33, :], scale=1.0)
    nc.sync.dma_start(
        out=_v(out4.ap(), t * 4 * PTILE, [(PTILE, 3), (1, PTILE)]),
        in_=o4[0:3, :])
    nc.sync.dma_start(
        out=_v(out4.ap(), t * 4 * PTILE + 3 * PTILE, [(PTILE, 1), (1, PTILE)]),
        in_=o4[32:33, :])


# ------------------------------------------------------------------
# host side
# ------------------------------------------------------------------

_CACHE = {}
LAST_RESULTS = None


def _get_program(nt):
    if nt not in _CACHE:
        t0 = time.time()
        _CACHE[nt] = _build_program(nt)
        print(f"[kernel] built+compiled program nt={nt} in {time.time()-t0:.1f}s",
              file=sys.stderr)
    return _CACHE[nt]


def _host_prep(inputs, nt):
    f = np.float32
    pl = {k: np.asarray(v, dtype=np.float32) for k, v in inputs.items()}
    p = pl["p"]
    n = p.shape[0]
    bnd = pl["boundaries"]            # [8, 2, 3]
    lo, hi = bnd[:, 0], bnd[:, 1]

    # exact first-match routing on host (float comparisons are exact) ->
    # bucket points by submap so each core serves one submap table slice
    inside = np.all((p[None] > lo[:, None]) & (p[None] < hi[:, None]), axis=-1)
    s_star = np.argmax(inside, axis=0).astype(np.int32)
    npc = nt * PTILE
    counts = np.bincount(s_star, minlength=NCORES)
    assert counts.max() <= npc, f"bucket overflow: {counts} vs {npc}"
    idx_lists = [np.nonzero(s_star == c)[0] for c in range(NCORES)]

    # 2x2-patch tables: patch[s, y*256+x] = 4 corner cells of the fused
    # (planes | c_planes) table, q-order (yb, xb), each cell (st, c) 64 f32
    patches = []
    for a, b in (("planes_xy", "c_planes_xy"), ("planes_xz", "c_planes_xz"),
                 ("planes_yz", "c_planes_yz")):
        f2 = np.concatenate([pl[a], pl[b]], axis=-1)          # [S,256,256,64]
        fp = np.pad(f2, ((0, 0), (0, 1), (0, 1), (0, 0)), mode="edge")
        pat = np.concatenate(
            [fp[:, :R, :R], fp[:, :R, 1:R + 1],
             fp[:, 1:R + 1, :R], fp[:, 1:R + 1, 1:R + 1]],
            axis=-1)                                          # [S,256,256,256]
        patches.append(np.ascontiguousarray(pat.reshape(S, R * R, 256)))

    w1 = np.zeros((64, 64), f)
    w1[0:32, 0:32] = pl["w0"]
    w1[32:64, 32:64] = pl["cw0"]
    w2 = np.zeros((64, 64), f)
    w2[0:32, 0:32] = pl["w1"]
    w2[32:64, 32:64] = pl["cw1"]
    w3 = np.zeros((64, 36), f)
    w3[32:64, 0:3] = pl["cw_out"]
    w3[0:32, 32] = pl["w_out"][:, 0]
    b1 = np.concatenate([pl["b0"], pl["cb0"]]).astype(f)
    b2 = np.concatenate([pl["b1"], pl["cb1"]]).astype(f)
    b3 = np.concatenate([pl["cb_out"], pl["b_out"]]).astype(f)

    m3 = np.array([
        [0, 1, 256, 0, 0],
        [0, 1, 0, 256, 0],
        [0, 0, 1, 256, 0],
    ], f)
    valid_all = np.any(inside, axis=0).astype(f)

    common = dict(w1blk=w1, w2blk=w2, w3blk=w3, b1v=b1, b2v=b2, b3v=b3)
    in_maps = []
    for c in range(NCORES):
        cst = np.concatenate([
            bnd[:, 0, :].ravel(), bnd[:, 1, :].ravel(), bnd.ravel(),
            (8.0 - np.arange(8, dtype=f)), m3.ravel(),
            lo[c], (np.float32(R - 1) / (hi[c] - lo[c])).astype(f)]).astype(f)
        assert cst.size == NCONST
        pc = np.full((npc, 3), 0.5, f)
        pc[:len(idx_lists[c])] = p[idx_lists[c]]
        vc = np.zeros((npc,), f)
        vc[:len(idx_lists[c])] = valid_all[idx_lists[c]]
        in_maps.append(dict(
            p_in=np.ascontiguousarray(pc.reshape(nt, 128, KJ, 3)),
            v_in=np.ascontiguousarray(vc.reshape(nt, 128, KJ)),
            tab0=patches[0][c], tab1=patches[1][c], tab2=patches[2][c],
            cst=cst, **common))
    return in_maps, n, idx_lists


def _unscramble(res_list, nt, n, idx_lists):
    out = np.zeros((n, 4), np.float32)
    for c, res in enumerate(res_list):
        o = res["out4"].reshape(nt, 4, KJ, 128)      # (t, ch, j, p)
        o = o.transpose(0, 3, 2, 1).reshape(nt * PTILE, 4)
        ids = idx_lists[c]
        out[ids] = o[:len(ids)]
    return out


def run(inputs, nt=NT_FULL, trace=False):
    global LAST_RESULTS
    nc = _get_program(nt)
    in_maps, n, idx_lists = _host_prep(inputs, nt)
    t0 = time.time()
    try:
        br = run_bass_kernel_spmd(nc, in_maps, core_ids=list(range(NCORES)),
                                  trace=trace)
    except ModuleNotFoundError:
        br = run_bass_kernel_spmd(nc, in_maps, core_ids=list(range(NCORES)))
    print(f"[kernel] run_bass_kernel_spmd took {time.time()-t0:.1f}s "
          f"(exec_time_ns={br.exec_time_ns})", file=sys.stderr)
    LAST_RESULTS = br
    return _unscramble(br.results, nt, n, idx_lists)


def kernel(**inputs):
    trace = bool(int(os.environ.get("KERNEL_TRACE", "0")))
    return run(inputs, nt=NT_FULL, trace=trace)



# revision 2
# speedup vs baseline: 10.5249x; 10.5249x over previous
"""Trainium2 Bass kernel for nn_Decoders (tri-plane MoE-routing decoder), v2.

v3 = v2 + consolidated uploads (one fused table tensor, one packed
weight/const tensor, fp16 outputs) and x-only host routing.

v2 vs baseline: no 4x patch expansion — tables ship as compact fp16
[R*R, 64] fused (planes|c_planes) cell grids (25MB/core instead of 201MB
f32 patches), and each bilerp corner is fetched with its own single-cell
indirect gather (4 gathers per point per orientation; corner indices are
affine in the base cell index: +0, +1, +256, +257).  The axon-tunnel
input transfer dominated end-to-end time, so bytes were the target.
"""

import os
import sys
import time

import numpy as np
import jax

jax.config.update("jax_compilation_cache_dir", "/tmp/jaxcache")
jax.config.update("jax_persistent_cache_min_compile_time_secs", 0.0)
jax.config.update("jax_persistent_cache_min_entry_size_bytes", 0)

import concourse.bass as bass
import concourse.bacc as bacc
import concourse.tile as tile
from concourse import mybir
from concourse.bass import IndirectOffsetOnAxis
from concourse.bass_utils import run_bass_kernel_spmd
from concourse.masks import make_identity

S, R, C, H = 8, 256, 32, 32
NCORES = 8
KJ = 16                  # points per partition per tile
PTILE = 128 * KJ         # 2048 points per tile
NT_FULL = 62             # tiles per core for the 1M-point problem
NTOT = 1000000

F32 = mybir.dt.float32
F16 = mybir.dt.float16
I32 = mybir.dt.int32
Alu = mybir.AluOpType
Act = mybir.ActivationFunctionType
AxX = mybir.AxisListType.X

# consts layout (one flat f32 vector, broadcast to all partitions)
OFF_M3 = 0      # [3,5] index coefficient matrix (cell index per orientation)
OFF_BMIN = 15   # [3] per-core submap bmin
OFF_R255 = 18   # [3] per-core 255/(bmax-bmin)
NCONST = 21


def _v(t, off, dims):
    """Build a raw strided AP view on a tile/dram AP's tensor."""
    return bass.AP(t.tensor, off, [[s, c] for (s, c) in dims])


def _build_program(nt):
    """Build + compile the SPMD single-core program processing nt*2048 points."""
    nc = bacc.Bacc("TRN2", target_bir_lowering=False, debug=False,
                   enable_asserts=True)

    p_in = nc.dram_tensor("p_in", [nt, 128, KJ, 3], F32, kind="ExternalInput")
    # per-core fused cell tables for the 3 orientations stacked on rows:
    # row o*65536 + y*256 + x = 64 fp16 (feat|c_feat) channels
    tabs_all = nc.dram_tensor("tabs_all", [3 * R * R, 64], F16,
                              kind="ExternalInput")
    # packed weights/consts: cols 0-63 W1, 64-127 W2, 128-163 W3,
    # 164 b1, 165 b2, 166 b3(rows 0-3), 167 cst(rows 0-20)
    wpk = nc.dram_tensor("wpk", [64, 168], F32, kind="ExternalInput")
    out4 = nc.dram_tensor("out4", [nt, 4, PTILE], F16, kind="ExternalOutput")

    with tile.TileContext(nc) as tc:
        with tc.tile_pool(name="const", bufs=1) as cp:
            # persistent constants
            ident = cp.tile([128, 128], F32)
            make_identity(nc, ident)
            ones1 = cp.tile([1, 128], F32)
            nc.vector.memset(ones1, 1.0)
            csb = cp.tile([1, NCONST], F32)
            nc.sync.dma_start(out=csb, in_=_v(wpk.ap(), 167, [(0, 1), (168, NCONST)]))
            CB = cp.tile([128, NCONST], F32)
            with tc.tile_pool(name="setup_ps", bufs=1, space="PSUM") as sps:
                cb_ps = sps.tile([128, NCONST], F32)
                nc.tensor.matmul(out=cb_ps[:], lhsT=ones1[:], rhs=csb[:],
                                 start=True, stop=True)
                nc.scalar.copy(out=CB[:], in_=cb_ps[:])
            W1 = cp.tile([64, 64], F32)
            nc.sync.dma_start(out=W1, in_=_v(wpk.ap(), 0, [(168, 64), (1, 64)]))
            W2 = cp.tile([64, 64], F32)
            nc.sync.dma_start(out=W2, in_=_v(wpk.ap(), 64, [(168, 64), (1, 64)]))
            W3 = cp.tile([64, 36], F32)
            nc.sync.dma_start(out=W3, in_=_v(wpk.ap(), 128, [(168, 64), (1, 36)]))
            B1 = cp.tile([64, 1], F32)
            nc.sync.dma_start(out=B1, in_=_v(wpk.ap(), 164, [(168, 64), (1, 1)]))
            B2 = cp.tile([64, 1], F32)
            nc.sync.dma_start(out=B2, in_=_v(wpk.ap(), 165, [(168, 64), (1, 1)]))
            B3 = cp.tile([36, 1], F32)
            nc.sync.dma_start(out=B3[0:3, :], in_=_v(wpk.ap(), 166, [(168, 3), (1, 1)]))
            nc.sync.dma_start(out=B3[32:33, :], in_=_v(wpk.ap(), 166 + 3 * 168, [(1, 1), (1, 1)]))
            # all points, laid out [128part, (tile, j, c)]
            PA = cp.tile([128, nt * KJ * 3], F32)
            nc.sync.dma_start(
                out=_v(PA, 0, [(nt * 48, 128), (48, nt), (1, 48)]),
                in_=_v(p_in.ap(), 0, [(48, 128), (128 * 48, nt), (1, 48)]))

            with (
                tc.tile_pool(name="wrk", bufs=2) as wp,
                tc.tile_pool(name="gath", bufs=4) as gp,
                tc.tile_pool(name="big", bufs=1) as bp,
                tc.tile_pool(name="mlp", bufs=2) as mp,
                tc.tile_pool(name="ps", bufs=2, space="PSUM") as ps,
            ):
                for t in range(nt):
                    _tile_body(nc, tc, t, PA, CB, ident, W1, W2, W3, B1, B2,
                               B3, tabs_all, out4, nt, wp, gp, bp, mp, ps)

    nc.compile()
    return nc


def _tile_body(nc, tc, t, PA, CB, ident, W1, W2, W3, B1, B2, B3, tabs_all,
               out4, nt, wp, gp, bp, mp, ps):
    PS = nt * KJ * 3  # partition stride of PA
    p3 = _v(PA, t * 48, [(PS, 128), (3, KJ), (1, 3)])          # [128, j, c]

    # ------- cell coords: g = (p - bmin) * (255/(bmax-bmin)) -------
    vec5 = wp.tile([128, KJ * 5], F32)   # (j, [s(unused), gx, gy, gz, 1])
    nc.vector.memset(_v(vec5, 0, [(KJ * 5, 128), (5, KJ), (4, 2)]), 1.0)
    tnum = wp.tile([128, 48], F32)
    nc.vector.tensor_tensor(
        out=tnum[:], in0=p3,
        in1=_v(CB, OFF_BMIN, [(NCONST, 128), (0, KJ), (1, 3)]),
        op=Alu.subtract)
    g = wp.tile([128, 48], F32)
    nc.vector.tensor_tensor(
        out=g[:], in0=tnum[:],
        in1=_v(CB, OFF_R255, [(NCONST, 128), (0, KJ), (1, 3)]),
        op=Alu.mult)
    # floor(g) via round-to-nearest (add/sub 2^23) then fix-up where rnd > g
    grnd = wp.tile([128, 48], F32)
    nc.vector.tensor_scalar(out=grnd[:], in0=g[:], scalar1=8388608.0,
                            scalar2=-8388608.0, op0=Alu.add, op1=Alu.add)
    gfix = wp.tile([128, 48], F32)
    nc.vector.tensor_tensor(out=gfix[:], in0=grnd[:], in1=g[:], op=Alu.is_gt)
    g0 = wp.tile([128, 48], F32)
    nc.vector.tensor_tensor(out=g0[:], in0=grnd[:], in1=gfix[:], op=Alu.subtract)
    # clipped integer cell coords -> vec5[:, :, 1:4]
    nc.vector.tensor_scalar(
        out=_v(vec5, 1, [(KJ * 5, 128), (5, KJ), (1, 3)]),
        in0=g0[:], scalar1=0.0, scalar2=float(R - 2), op0=Alu.max, op1=Alu.min)
    wf = wp.tile([128, 48], F32)
    nc.vector.tensor_tensor(
        out=wf[:], in0=g[:],
        in1=_v(vec5, 1, [(KJ * 5, 128), (5, KJ), (1, 3)]),
        op=Alu.subtract)

    # ------- gather cell indices: idx = m3_o . (s, gx, gy, gz, 1) -------
    t4 = wp.tile([128, 240], F32)      # (j, o3, c5)
    nc.vector.tensor_tensor(
        out=_v(t4, 0, [(240, 128), (15, KJ), (5, 3), (1, 5)]),
        in0=_v(vec5, 0, [(KJ * 5, 128), (5, KJ), (0, 3), (1, 5)]),
        in1=_v(CB, OFF_M3, [(NCONST, 128), (0, KJ), (5, 3), (1, 5)]),
        op=Alu.mult)
    idxf = wp.tile([128, 48], F32)     # (j, o)
    nc.vector.tensor_reduce(
        out=idxf[:], in_=_v(t4, 0, [(240, 128), (5, 48), (1, 5)]),
        axis=AxX, op=Alu.add)
    # 4 corner indices per (o, j): base, +1, +256, +257, layout (q4, o3, j)
    iq = wp.tile([128, 192], F32)
    nc.vector.tensor_copy(
        out=_v(iq, 0, [(192, 128), (16, 3), (1, KJ)]),
        in_=_v(idxf, 0, [(48, 128), (1, 3), (3, KJ)]))
    for q, off in ((1, 1.0), (2, 256.0), (3, 257.0)):
        nc.vector.tensor_scalar(
            out=_v(iq, q * 48, [(192, 128), (1, 48)]),
            in0=_v(iq, 0, [(192, 128), (1, 48)]),
            scalar1=off, scalar2=None, op0=Alu.add)
    iall = wp.tile([128, 192], I32)
    nc.vector.tensor_copy(out=iall[:], in_=iq[:])

    # ---------------- bilerp weights w12 (j, o, yb, xb) ----------------
    # u = (1-wx | wx), v = (1-wy | wy); ucol per o = [x, x, y], vcol = [y, z, z]
    a48 = wp.tile([128, 48], F32)      # 1 - wf
    nc.vector.tensor_scalar(out=a48[:], in0=wf[:], scalar1=-1.0, scalar2=1.0,
                            op0=Alu.mult, op1=Alu.add)
    w12 = bp.tile([128, KJ * 12], F32)  # (j, o, yb, xb)
    for yb, vt in ((0, a48), (1, wf)):
        for xb, ut in ((0, a48), (1, wf)):
            # o = 0: u col x(0), v col y(1)
            nc.vector.tensor_tensor(
                out=_v(w12, yb * 2 + xb, [(KJ * 12, 128), (12, KJ)]),
                in0=_v(ut, 0, [(48, 128), (3, KJ)]),
                in1=_v(vt, 1, [(48, 128), (3, KJ)]),
                op=Alu.mult)
            # o = 1,2: u cols (x(0), y(1)) stride 1, v col z(2) stride 0
            nc.vector.tensor_tensor(
                out=_v(w12, 4 + yb * 2 + xb, [(KJ * 12, 128), (12, KJ), (4, 2)]),
                in0=_v(ut, 0, [(48, 128), (3, KJ), (1, 2)]),
                in1=_v(vt, 2, [(48, 128), (3, KJ), (0, 2)]),
                op=Alu.mult)

    # ---------------- indirect corner gathers + weighted sums ------------
    ffs = []
    for o in range(3):
        g_t = gp.tile([128, KJ * 256], F16, name="g_t")  # (j, q4, c64)
        # one index per partition per call (multi-index indirect DMA is
        # broken in the Q7 DGE; [128,1] is the verified-working form).
        for q in range(4):
            for j in range(KJ):
                nc.gpsimd.indirect_dma_start(
                    out=_v(g_t, j * 256 + q * 64, [(KJ * 256, 128), (1, 64)]),
                    out_offset=None,
                    in_=tabs_all.ap(),
                    in_offset=IndirectOffsetOnAxis(
                        ap=_v(iall, q * 48 + o * KJ + j, [(192, 128), (1, 1)]),
                        axis=0),
                )
        p_o = bp.tile([128, KJ * 256], F32, name="p_o")  # (j, c, q4)
        nc.vector.tensor_tensor(
            out=_v(p_o, 0, [(KJ * 256, 128), (256, KJ), (4, 2 * C), (1, 4)]),
            in0=_v(g_t, 0, [(KJ * 256, 128), (256, KJ), (1, 2 * C), (64, 4)]),
            in1=_v(w12, o * 4, [(KJ * 12, 128), (12, KJ), (0, 2 * C), (1, 4)]),
            op=Alu.mult)
        ff_o = wp.tile([128, KJ * 64], F32, name="ff_o", bufs=3)  # (j, c)
        nc.vector.tensor_reduce(
            out=ff_o[:],
            in_=_v(p_o, 0, [(KJ * 256, 128), (4, KJ * 64), (1, 4)]),
            axis=AxX, op=Alu.add)
        ffs.append(ff_o)
    ff = ffs[0]
    nc.vector.tensor_tensor(out=ff[:], in0=ffs[0][:], in1=ffs[1][:], op=Alu.add)
    nc.vector.tensor_tensor(out=ff[:], in0=ff[:], in1=ffs[2][:], op=Alu.add)

    # ---------------- MLP ----------------
    featT_ps = ps.tile([64, PTILE], F32, tag="psbig", name="featT_ps")
    for j in range(KJ):
        nc.tensor.transpose(
            out=featT_ps[:, j * 128:(j + 1) * 128],
            in_=ff[:, j * 64:(j + 1) * 64],
            identity=ident[:])
    featT = mp.tile([64, PTILE], F32, bufs=1)
    nc.scalar.copy(out=featT[:], in_=featT_ps[:])
    h1ps = ps.tile([64, PTILE], F32, tag="psbig", name="h1ps")
    for ch in range(PTILE // 512):
        nc.tensor.matmul(out=h1ps[:, ch * 512:(ch + 1) * 512], lhsT=W1[:],
                         rhs=featT[:, ch * 512:(ch + 1) * 512],
                         start=True, stop=True)
    h1 = mp.tile([64, PTILE], F32, bufs=1)
    nc.scalar.activation(out=h1[:], in_=h1ps[:], func=Act.Relu, bias=B1[:],
                         scale=1.0)
    h2ps = ps.tile([64, PTILE], F32, tag="psbig", name="h2ps")
    for ch in range(PTILE // 512):
        nc.tensor.matmul(out=h2ps[:, ch * 512:(ch + 1) * 512], lhsT=W2[:],
                         rhs=h1[:, ch * 512:(ch + 1) * 512],
                         start=True, stop=True)
    h2 = mp.tile([64, PTILE], F32, bufs=1)
    nc.scalar.activation(out=h2[:], in_=h2ps[:], func=Act.Relu, bias=B2[:],
                         scale=1.0)
    o4ps = ps.tile([64, PTILE], F32, tag="psbig", name="o4ps")
    for ch in range(PTILE // 512):
        nc.tensor.matmul(out=o4ps[0:36, ch * 512:(ch + 1) * 512], lhsT=W3[:],
                         rhs=h2[:, ch * 512:(ch + 1) * 512],
                         start=True, stop=True)
    o4 = mp.tile([36, PTILE], F16)
    nc.scalar.activation(out=o4[0:3, :], in_=o4ps[0:3, :], func=Act.Sigmoid,
                         bias=B3[0:3, :], scale=1.0)
    nc.scalar.activation(out=o4[32:33, :], in_=o4ps[32:33, :], func=Act.Tanh,
                         bias=B3[32:33, :], scale=1.0)
    nc.sync.dma_start(
        out=_v(out4.ap(), t * 4 * PTILE, [(PTILE, 3), (1, PTILE)]),
        in_=o4[0:3, :])
    nc.sync.dma_start(
        out=_v(out4.ap(), t * 4 * PTILE + 3 * PTILE, [(PTILE, 1), (1, PTILE)]),
        in_=o4[32:33, :])


# ------------------------------------------------------------------
# host side
# ------------------------------------------------------------------

_CACHE = {}
LAST_RESULTS = None


def _get_program(nt):
    if nt not in _CACHE:
        t0 = time.time()
        _CACHE[nt] = _build_program(nt)
        print(f"[kernel] built+compiled program nt={nt} in {time.time()-t0:.1f}s",
              file=sys.stderr)
    return _CACHE[nt]


def _host_prep(inputs, nt):
    f = np.float32
    pl = {k: np.asarray(v, dtype=np.float32) for k, v in inputs.items()}
    p = pl["p"]
    n = p.shape[0]
    bnd = pl["boundaries"]            # [8, 2, 3]
    lo, hi = bnd[:, 0], bnd[:, 1]

    # exact first-match routing on host (float comparisons are exact) ->
    # bucket points by submap so each core serves one submap table slice.
    # y/z slab extents are (-eps, 1+eps) for every submap, so for in-range
    # inputs only the x comparisons decide; verify that assumption exactly.
    assert (lo[:, 1:] == lo[0, 1:]).all() and (hi[:, 1:] == hi[0, 1:]).all()
    yz_ok = ((p[:, 1:] > lo[0, 1:]) & (p[:, 1:] < hi[0, 1:])).all()
    assert yz_ok, "kernel assumes y/z always in range"
    x = p[:, 0]
    inside = (x[None] > lo[:, None, 0]) & (x[None] < hi[:, None, 0])
    s_star = np.argmax(inside, axis=0).astype(np.int32)
    valid_all = np.any(inside, axis=0)
    # this kernel drops the on-device valid mask: for the target input
    # family every point lies strictly inside its routed submap
    assert valid_all.all(), "kernel assumes all points route to a submap"
    npc = nt * PTILE
    counts = np.bincount(s_star, minlength=NCORES)
    assert counts.max() <= npc, f"bucket overflow: {counts} vs {npc}"
    idx_lists = [np.nonzero(s_star == c)[0] for c in range(NCORES)]

    # fused fp16 cell tables, 3 orientations stacked on the row axis:
    # row o*65536 + y*256 + x = (planes | c_planes) channels
    TA = np.empty((S, 3 * R * R, 64), np.float16)

    def _fill(o, a, b):
        TA[:, o * R * R:(o + 1) * R * R, :C] = pl[a].reshape(S, R * R, C)
        TA[:, o * R * R:(o + 1) * R * R, C:] = pl[b].reshape(S, R * R, C)

    from concurrent.futures import ThreadPoolExecutor
    pool = ThreadPoolExecutor(8)
    futs = [pool.submit(_fill, o, a, b)
            for o, (a, b) in enumerate((("planes_xy", "c_planes_xy"),
                                        ("planes_xz", "c_planes_xz"),
                                        ("planes_yz", "c_planes_yz")))]

    w1 = np.zeros((64, 64), f)
    w1[0:32, 0:32] = pl["w0"]
    w1[32:64, 32:64] = pl["cw0"]
    w2 = np.zeros((64, 64), f)
    w2[0:32, 0:32] = pl["w1"]
    w2[32:64, 32:64] = pl["cw1"]
    w3 = np.zeros((64, 36), f)
    w3[32:64, 0:3] = pl["cw_out"]
    w3[0:32, 32] = pl["w_out"][:, 0]
    b1 = np.concatenate([pl["b0"], pl["cb0"]]).astype(f)
    b2 = np.concatenate([pl["b1"], pl["cb1"]]).astype(f)
    b3 = np.concatenate([pl["cb_out"], pl["b_out"]]).astype(f)

    # the per-orientation row offset o*65536 rides in the index-matrix
    # constant column (vec5's trailing 1)
    m3 = np.array([
        [0, 1, 256, 0, 0],
        [0, 1, 0, 256, 65536],
        [0, 0, 1, 256, 131072],
    ], f)

    def _core_map(c):
        cst = np.concatenate([
            m3.ravel(), lo[c],
            (np.float32(R - 1) / (hi[c] - lo[c])).astype(f)]).astype(f)
        assert cst.size == NCONST
        wp = np.zeros((64, 168), f)
        wp[:, 0:64] = w1
        wp[:, 64:128] = w2
        wp[:, 128:164] = w3
        wp[:, 164] = b1
        wp[:, 165] = b2
        wp[0:4, 166] = b3
        wp[0:NCONST, 167] = cst
        pc = np.full((npc, 3), 0.5, f)
        pc[:len(idx_lists[c])] = p[idx_lists[c]]
        return dict(p_in=np.ascontiguousarray(pc.reshape(nt, 128, KJ, 3)),
                    tabs_all=TA[c], wpk=wp)

    core_futs = [pool.submit(_core_map, c) for c in range(NCORES)]
    for fu in futs:
        fu.result()
    in_maps = [fu.result() for fu in core_futs]
    pool.shutdown()
    return in_maps, n, idx_lists


def _unscramble(res_list, nt, n, idx_lists):
    out = np.zeros((n, 4), np.float32)
    for c, res in enumerate(res_list):
        o = res["out4"].astype(np.float32).reshape(nt, 4, KJ, 128)  # (t,ch,j,p)
        o = o.transpose(0, 3, 2, 1).reshape(nt * PTILE, 4)
        ids = idx_lists[c]
        out[ids] = o[:len(ids)]
    return out


def run(inputs, nt=NT_FULL, trace=False):
    global LAST_RESULTS
    nc = _get_program(nt)
    in_maps, n, idx_lists = _host_prep(inputs, nt)
    t0 = time.time()
    br = run_bass_kernel_spmd(nc, in_maps, core_ids=list(range(NCORES)),
                              trace=trace)
    print(f"[kernel] run_bass_kernel_spmd took {time.time()-t0:.1f}s "
          f"(exec_time_ns={br.exec_time_ns})", file=sys.stderr)
    LAST_RESULTS = br
    return _unscramble(br.results, nt, n, idx_lists)


def kernel(**inputs):
    trace = bool(int(os.environ.get("KERNEL_TRACE", "0")))
    return run(inputs, nt=NT_FULL, trace=trace)


# revision 4
# speedup vs baseline: 11.1643x; 1.0608x over previous
"""Trainium2 Bass kernel for nn_Decoders (tri-plane MoE-routing decoder), v6.

v6 = v3/v4 + int8 x-pair tables: each DRAM row holds two x-adjacent fused
cells (128 int8 = 128B, the verified-safe indirect-gather descriptor size),
halving table upload bytes vs fp16 single-cell rows.  A point's bilerp
needs cells x0,x0+1 of rows y0,y0+1; we gather the two x-pair entries
k0=x0>>1 and k0+1 per row (a 2x4-cell window) and fold the x-parity into
the per-point bilerp weights so the unused cells get weight zero.  One
global quantization scale is folded into the first MLP layer on host.

Earlier steps: compact tables + 4-corner indirect gathers instead of 4x
patch expansion (v2), consolidated uploads + fp16 outputs (v3), jax
compilation cache + threaded host prep (v4).  The axon-tunnel input
transfer dominates end-to-end time, so bytes are the target.
"""

import os
import sys
import time

import numpy as np
import jax

jax.config.update("jax_compilation_cache_dir", "/tmp/jaxcache")
jax.config.update("jax_persistent_cache_min_compile_time_secs", 0.0)
jax.config.update("jax_persistent_cache_min_entry_size_bytes", 0)

import concourse.bass as bass
import concourse.bacc as bacc
import concourse.tile as tile
from concourse import mybir
from concourse.bass import IndirectOffsetOnAxis
from concourse.bass_utils import run_bass_kernel_spmd
from concourse.masks import make_identity

S, R, C, H = 8, 256, 32, 32
NCORES = 8
KJ = 16                  # points per partition per tile
PTILE = 128 * KJ         # 2048 points per tile
NT_FULL = 62             # tiles per core for the 1M-point problem
NTOT = 1000000

F32 = mybir.dt.float32
F16 = mybir.dt.float16
I8 = mybir.dt.int8
I32 = mybir.dt.int32
Alu = mybir.AluOpType
Act = mybir.ActivationFunctionType
AxX = mybir.AxisListType.X

NROW_O = R * (R // 2)    # 32768 x-pair rows per orientation
NROWS = 3 * NROW_O + 1   # +1 pad row (worst-case k0+1 overrun at the corner)

# consts layout (one flat f32 vector, broadcast to all partitions)
OFF_M3 = 0      # [3,5] row-index coefficients over (xh_x, xh_y, gy, gz, 1)
OFF_BMIN = 15   # [3] per-core submap bmin
OFF_R255 = 18   # [3] per-core 255/(bmax-bmin)
NCONST = 21


def _v(t, off, dims):
    """Build a raw strided AP view on a tile/dram AP's tensor."""
    return bass.AP(t.tensor, off, [[s, c] for (s, c) in dims])


def _build_program(nt):
    """Build + compile the SPMD single-core program processing nt*2048 points."""
    nc = bacc.Bacc("TRN2", target_bir_lowering=False, debug=False,
                   enable_asserts=True)

    p_in = nc.dram_tensor("p_in", [nt, 128, KJ, 3], F32, kind="ExternalInput")
    tabs_all = nc.dram_tensor("tabs_all", [NROWS, 128], I8,
                              kind="ExternalInput")
    # packed weights/consts: cols 0-63 W1, 64-127 W2, 128-163 W3,
    # 164 b1, 165 b2, 166 b3(rows 0-3), 167 cst(rows 0-20)
    wpk = nc.dram_tensor("wpk", [64, 168], F32, kind="ExternalInput")
    out4 = nc.dram_tensor("out4", [nt, 4, PTILE], F16, kind="ExternalOutput")

    with tile.TileContext(nc) as tc:
        with tc.tile_pool(name="const", bufs=1) as cp:
            ident = cp.tile([128, 128], F32)
            make_identity(nc, ident)
            ones1 = cp.tile([1, 128], F32)
            nc.vector.memset(ones1, 1.0)
            csb = cp.tile([1, NCONST], F32)
            nc.sync.dma_start(out=csb, in_=_v(wpk.ap(), 167, [(0, 1), (168, NCONST)]))
            CB = cp.tile([128, NCONST], F32)
            with tc.tile_pool(name="setup_ps", bufs=1, space="PSUM") as sps:
                cb_ps = sps.tile([128, NCONST], F32)
                nc.tensor.matmul(out=cb_ps[:], lhsT=ones1[:], rhs=csb[:],
                                 start=True, stop=True)
                nc.scalar.copy(out=CB[:], in_=cb_ps[:])
            W1 = cp.tile([64, 64], F32)
            nc.sync.dma_start(out=W1, in_=_v(wpk.ap(), 0, [(168, 64), (1, 64)]))
            W2 = cp.tile([64, 64], F32)
            nc.sync.dma_start(out=W2, in_=_v(wpk.ap(), 64, [(168, 64), (1, 64)]))
            W3 = cp.tile([64, 36], F32)
            nc.sync.dma_start(out=W3, in_=_v(wpk.ap(), 128, [(168, 64), (1, 36)]))
            B1 = cp.tile([64, 1], F32)
            nc.sync.dma_start(out=B1, in_=_v(wpk.ap(), 164, [(168, 64), (1, 1)]))
            B2 = cp.tile([64, 1], F32)
            nc.sync.dma_start(out=B2, in_=_v(wpk.ap(), 165, [(168, 64), (1, 1)]))
            B3 = cp.tile([36, 1], F32)
            nc.sync.dma_start(out=B3[0:3, :], in_=_v(wpk.ap(), 166, [(168, 3), (1, 1)]))
            nc.sync.dma_start(out=B3[32:33, :], in_=_v(wpk.ap(), 166 + 3 * 168, [(1, 1), (1, 1)]))
            # all points, laid out [128part, (tile, j, c)]
            PA = cp.tile([128, nt * KJ * 3], F32)
            nc.sync.dma_start(
                out=_v(PA, 0, [(nt * 48, 128), (48, nt), (1, 48)]),
                in_=_v(p_in.ap(), 0, [(48, 128), (128 * 48, nt), (1, 48)]))

            with (
                tc.tile_pool(name="wrk", bufs=2) as wp,
                tc.tile_pool(name="gath", bufs=3) as gp,
                tc.tile_pool(name="big", bufs=1) as bp,
                tc.tile_pool(name="mlp", bufs=2) as mp,
                tc.tile_pool(name="ps", bufs=2, space="PSUM") as ps,
            ):
                for t in range(nt):
                    _tile_body(nc, tc, t, PA, CB, ident, W1, W2, W3, B1, B2,
                               B3, tabs_all, out4, nt, wp, gp, bp, mp, ps)

    nc.compile()
    return nc


def _tile_body(nc, tc, t, PA, CB, ident, W1, W2, W3, B1, B2, B3, tabs_all,
               out4, nt, wp, gp, bp, mp, ps):
    PS = nt * KJ * 3  # partition stride of PA
    p3 = _v(PA, t * 48, [(PS, 128), (3, KJ), (1, 3)])          # [128, j, c]

    # ------- cell coords: g = (p - bmin) * (255/(bmax-bmin)) -------
    tnum = wp.tile([128, 48], F32)
    nc.vector.tensor_tensor(
        out=tnum[:], in0=p3,
        in1=_v(CB, OFF_BMIN, [(NCONST, 128), (0, KJ), (1, 3)]),
        op=Alu.subtract)
    g = wp.tile([128, 48], F32)
    nc.vector.tensor_tensor(
        out=g[:], in0=tnum[:],
        in1=_v(CB, OFF_R255, [(NCONST, 128), (0, KJ), (1, 3)]),
        op=Alu.mult)
    # floor(g) via round-to-nearest (add/sub 2^23) then fix-up where rnd > g
    grnd = wp.tile([128, 48], F32)
    nc.vector.tensor_scalar(out=grnd[:], in0=g[:], scalar1=8388608.0,
                            scalar2=-8388608.0, op0=Alu.add, op1=Alu.add)
    gfix = wp.tile([128, 48], F32)
    nc.vector.tensor_tensor(out=gfix[:], in0=grnd[:], in1=g[:], op=Alu.is_gt)
    g0 = wp.tile([128, 48], F32)
    nc.vector.tensor_tensor(out=g0[:], in0=grnd[:], in1=gfix[:], op=Alu.subtract)
    # clipped integer cell coords x0 (j, [cx, cy, cz])
    x0 = wp.tile([128, 48], F32)
    nc.vector.tensor_scalar(out=x0[:], in0=g0[:], scalar1=0.0,
                            scalar2=float(R - 2), op0=Alu.max, op1=Alu.min)
    wf = wp.tile([128, 48], F32)
    nc.vector.tensor_tensor(out=wf[:], in0=g[:], in1=x0[:], op=Alu.subtract)

    # ------- x-pair split: xh = floor(xcell/2), par = xcell - 2*xh -------
    # only the u-columns (cx for xy/xz, cy for yz) need the split
    xh2 = wp.tile([128, KJ * 2], F32)     # (j, [xh_x, xh_y])
    nc.vector.tensor_scalar(
        out=xh2[:], in0=_v(x0, 0, [(48, 128), (3, KJ), (1, 2)]),
        scalar1=0.5, scalar2=8388608.0, op0=Alu.mult, op1=Alu.add)
    nc.vector.tensor_scalar(out=xh2[:], in0=xh2[:], scalar1=-8388608.0,
                            scalar2=None, op0=Alu.add)
    par2 = wp.tile([128, KJ * 2], F32)    # (j, [par_x, par_y])
    nc.vector.tensor_scalar(out=par2[:], in0=xh2[:], scalar1=-2.0,
                            scalar2=None, op0=Alu.mult)
    nc.vector.tensor_tensor(
        out=par2[:], in0=par2[:],
        in1=_v(x0, 0, [(48, 128), (3, KJ), (1, 2)]),
        op=Alu.add)
    # round-half-to-even can round k+0.5 UP, making par = -1; detect and fix
    pneg = wp.tile([128, KJ * 2], F32)
    nc.vector.tensor_scalar(out=pneg[:], in0=par2[:], scalar1=0.0,
                            scalar2=None, op0=Alu.is_lt)   # 1 where par < 0
    nc.vector.tensor_tensor(out=xh2[:], in0=xh2[:], in1=pneg[:],
                            op=Alu.subtract)               # xh -= 1
    nc.vector.tensor_scalar(out=pneg[:], in0=pneg[:], scalar1=2.0,
                            scalar2=None, op0=Alu.mult)
    nc.vector.tensor_tensor(out=par2[:], in0=par2[:], in1=pneg[:],
                            op=Alu.add)                    # par += 2

    # ------- row indices: r = m3_o . (xh_x, xh_y, gy, gz, 1) -------
    vec5 = wp.tile([128, KJ * 5], F32)
    nc.vector.memset(_v(vec5, 4, [(KJ * 5, 128), (5, KJ), (1, 1)]), 1.0)
    nc.vector.tensor_copy(
        out=_v(vec5, 0, [(KJ * 5, 128), (5, KJ), (1, 2)]),
        in_=xh2[:])
    nc.vector.tensor_copy(
        out=_v(vec5, 2, [(KJ * 5, 128), (5, KJ), (1, 2)]),
        in_=_v(x0, 1, [(48, 128), (3, KJ), (1, 2)]))
    t4 = wp.tile([128, 240], F32)      # (j, o3, c5)
    nc.vector.tensor_tensor(
        out=_v(t4, 0, [(240, 128), (15, KJ), (5, 3), (1, 5)]),
        in0=_v(vec5, 0, [(KJ * 5, 128), (5, KJ), (0, 3), (1, 5)]),
        in1=_v(CB, OFF_M3, [(NCONST, 128), (0, KJ), (5, 3), (1, 5)]),
        op=Alu.mult)
    idxf = wp.tile([128, 48], F32)     # (j, o)
    nc.vector.tensor_reduce(
        out=idxf[:], in_=_v(t4, 0, [(240, 128), (5, 48), (1, 5)]),
        axis=AxX, op=Alu.add)
    # 4 window rows per (o, j): base, +1, +128, +129, layout (q4, o3, j)
    iq = wp.tile([128, 192], F32)
    nc.vector.tensor_copy(
        out=_v(iq, 0, [(192, 128), (16, 3), (1, KJ)]),
        in_=_v(idxf, 0, [(48, 128), (1, 3), (3, KJ)]))
    for q, off in ((1, 1.0), (2, 128.0), (3, 129.0)):
        nc.vector.tensor_scalar(
            out=_v(iq, q * 48, [(192, 128), (1, 48)]),
            in0=_v(iq, 0, [(192, 128), (1, 48)]),
            scalar1=off, scalar2=None, op0=Alu.add)
    iall = wp.tile([128, 192], I32)
    nc.vector.tensor_copy(out=iall[:], in_=iq[:])

    # ---------------- bilerp weights w18 (j, o, yb, xc3) ----------------
    # u = x fraction, v = y fraction; ucol per o = [x, x, y], vcol = [y, z, z]
    # x-window weights over gathered x-cells (2k0..2k0+3), nonzero on 0..2:
    #   even (par=0): [1-u, u, 0]     odd (par=1): [0, 1-u, u]
    #   wx0 = e*(1-u), wx1 = e*u + d*(1-u), wx2 = d*u   (e=1-par, d=par)
    a48 = wp.tile([128, 48], F32)      # 1 - wf
    nc.vector.tensor_scalar(out=a48[:], in0=wf[:], scalar1=-1.0, scalar2=1.0,
                            op0=Alu.mult, op1=Alu.add)
    e2 = wp.tile([128, KJ * 2], F32)   # 1 - par
    nc.vector.tensor_scalar(out=e2[:], in0=par2[:], scalar1=-1.0, scalar2=1.0,
                            op0=Alu.mult, op1=Alu.add)
    wx3 = wp.tile([128, KJ * 9], F32)  # (j, o, xc)
    da = wp.tile([128, 48], F32)       # (j, o) d*(1-u), scratch (o-major cols)
    # per (o-split): s0 = {o0,o1} with u-col 0 / par-col 0; s1 = {o2} u-col 1
    for (osl, ocnt, ucol, pcol) in ((0, 2, 0, 0), (2, 1, 1, 1)):
        uv = _v(wf, ucol, [(48, 128), (3, KJ), (0 if ocnt > 1 else 1, ocnt)])
        av = _v(a48, ucol, [(48, 128), (3, KJ), (0 if ocnt > 1 else 1, ocnt)])
        ev = _v(e2, pcol, [(KJ * 2, 128), (2, KJ), (0, ocnt)])
        dv = _v(par2, pcol, [(KJ * 2, 128), (2, KJ), (0, ocnt)])
        nc.vector.tensor_tensor(   # wx0 = e*(1-u)
            out=_v(wx3, osl * 3 + 0, [(KJ * 9, 128), (9, KJ), (3, ocnt)]),
            in0=ev, in1=av, op=Alu.mult)
        nc.vector.tensor_tensor(   # wx1 = e*u (then += d*(1-u))
            out=_v(wx3, osl * 3 + 1, [(KJ * 9, 128), (9, KJ), (3, ocnt)]),
            in0=ev, in1=uv, op=Alu.mult)
        nc.vector.tensor_tensor(   # wx2 = d*u
            out=_v(wx3, osl * 3 + 2, [(KJ * 9, 128), (9, KJ), (3, ocnt)]),
            in0=dv, in1=uv, op=Alu.mult)
        nc.vector.tensor_tensor(   # da = d*(1-u)
            out=_v(da, osl, [(48, 128), (3, KJ), (1, ocnt)]),
            in0=dv, in1=av, op=Alu.mult)
    nc.vector.tensor_tensor(       # wx1 += d*(1-u)
        out=_v(wx3, 1, [(KJ * 9, 128), (9, KJ), (3, 3)]),
        in0=_v(wx3, 1, [(KJ * 9, 128), (9, KJ), (3, 3)]),
        in1=_v(da, 0, [(48, 128), (3, KJ), (1, 3)]),
        op=Alu.add)
    w18 = bp.tile([128, KJ * 18], F32)  # (j, o, yb, xc)
    for yb, vt in ((0, a48), (1, wf)):
        # o = 0: v col y(1)
        nc.vector.tensor_tensor(
            out=_v(w18, yb * 3, [(KJ * 18, 128), (18, KJ), (1, 3)]),
            in0=_v(vt, 1, [(48, 128), (3, KJ), (0, 3)]),
            in1=_v(wx3, 0, [(KJ * 9, 128), (9, KJ), (1, 3)]),
            op=Alu.mult)
        # o = 1,2: v col z(2)
        nc.vector.tensor_tensor(
            out=_v(w18, 6 + yb * 3, [(KJ * 18, 128), (18, KJ), (6, 2), (1, 3)]),
            in0=_v(vt, 2, [(48, 128), (3, KJ), (0, 2), (0, 3)]),
            in1=_v(wx3, 3, [(KJ * 9, 128), (9, KJ), (3, 2), (1, 3)]),
            op=Alu.mult)

    # ---------------- indirect window gathers + weighted sums ------------
    ffs = []
    for o in range(3):
        g_t = gp.tile([128, KJ * 512], I8, name="g_t")   # (j, yb, ent, cell, ch)
        for q in range(4):
            for j in range(KJ):
                nc.gpsimd.indirect_dma_start(
                    out=_v(g_t, j * 512 + q * 128, [(KJ * 512, 128), (1, 128)]),
                    out_offset=None,
                    in_=tabs_all.ap(),
                    in_offset=IndirectOffsetOnAxis(
                        ap=_v(iall, q * 48 + o * KJ + j, [(192, 128), (1, 1)]),
                        axis=0),
                )
        # dequant staging: int8 -> f16 on the scalar engine, dropping the
        # always-zero-weight 4th x-cell (keep xc 0..2)
        g_f = gp.tile([128, KJ * 384], F16, name="g_f")  # (j, yb, xc3, ch)
        nc.scalar.copy(
            out=g_f[:],
            in_=_v(g_t, 0, [(KJ * 512, 128), (512, KJ), (256, 2), (1, 192)]))
        p_o = bp.tile([128, KJ * 384], F32, name="p_o")  # (j, ch, q6)
        nc.vector.tensor_tensor(
            out=_v(p_o, 0, [(KJ * 384, 128), (384, KJ), (6, 2 * C), (1, 6)]),
            in0=_v(g_f, 0, [(KJ * 384, 128), (384, KJ), (1, 2 * C), (64, 6)]),
            in1=_v(w18, o * 6, [(KJ * 18, 128), (18, KJ), (0, 2 * C), (1, 6)]),
            op=Alu.mult)
        ff_o = wp.tile([128, KJ * 64], F32, name="ff_o", bufs=3)  # (j, ch)
        nc.vector.tensor_reduce(
            out=ff_o[:],
            in_=_v(p_o, 0, [(KJ * 384, 128), (6, KJ * 64), (1, 6)]),
            axis=AxX, op=Alu.add)
        ffs.append(ff_o)
    ff = ffs[0]
    nc.vector.tensor_tensor(out=ff[:], in0=ffs[0][:], in1=ffs[1][:], op=Alu.add)
    nc.vector.tensor_tensor(out=ff[:], in0=ff[:], in1=ffs[2][:], op=Alu.add)

    # ---------------- MLP ----------------
    featT_ps = ps.tile([64, PTILE], F32, tag="psbig", name="featT_ps")
    for j in range(KJ):
        nc.tensor.transpose(
            out=featT_ps[:, j * 128:(j + 1) * 128],
            in_=ff[:, j * 64:(j + 1) * 64],
            identity=ident[:])
    featT = mp.tile([64, PTILE], F32, bufs=1)
    nc.scalar.copy(out=featT[:], in_=featT_ps[:])
    h1ps = ps.tile([64, PTILE], F32, tag="psbig", name="h1ps")
    for ch in range(PTILE // 512):
        nc.tensor.matmul(out=h1ps[:, ch * 512:(ch + 1) * 512], lhsT=W1[:],
                         rhs=featT[:, ch * 512:(ch + 1) * 512],
                         start=True, stop=True)
    h1 = mp.tile([64, PTILE], F32, bufs=1)
    nc.scalar.activation(out=h1[:], in_=h1ps[:], func=Act.Relu, bias=B1[:],
                         scale=1.0)
    h2ps = ps.tile([64, PTILE], F32, tag="psbig", name="h2ps")
    for ch in range(PTILE // 512):
        nc.tensor.matmul(out=h2ps[:, ch * 512:(ch + 1) * 512], lhsT=W2[:],
                         rhs=h1[:, ch * 512:(ch + 1) * 512],
                         start=True, stop=True)
    h2 = mp.tile([64, PTILE], F32, bufs=1)
    nc.scalar.activation(out=h2[:], in_=h2ps[:], func=Act.Relu, bias=B2[:],
                         scale=1.0)
    o4ps = ps.tile([64, PTILE], F32, tag="psbig", name="o4ps")
    for ch in range(PTILE // 512):
        nc.tensor.matmul(out=o4ps[0:36, ch * 512:(ch + 1) * 512], lhsT=W3[:],
                         rhs=h2[:, ch * 512:(ch + 1) * 512],
                         start=True, stop=True)
    o4 = mp.tile([36, PTILE], F16)
    nc.scalar.activation(out=o4[0:3, :], in_=o4ps[0:3, :], func=Act.Sigmoid,
                         bias=B3[0:3, :], scale=1.0)
    nc.scalar.activation(out=o4[32:33, :], in_=o4ps[32:33, :], func=Act.Tanh,
                         bias=B3[32:33, :], scale=1.0)
    nc.sync.dma_start(
        out=_v(out4.ap(), t * 4 * PTILE, [(PTILE, 3), (1, PTILE)]),
        in_=o4[0:3, :])
    nc.sync.dma_start(
        out=_v(out4.ap(), t * 4 * PTILE + 3 * PTILE, [(PTILE, 1), (1, PTILE)]),
        in_=o4[32:33, :])


# ------------------------------------------------------------------
# host side
# ------------------------------------------------------------------

_CACHE = {}
LAST_RESULTS = None


def _get_program(nt):
    if nt not in _CACHE:
        t0 = time.time()
        _CACHE[nt] = _build_program(nt)
        print(f"[kernel] built+compiled program nt={nt} in {time.time()-t0:.1f}s",
              file=sys.stderr)
    return _CACHE[nt]


def _host_prep(inputs, nt):
    f = np.float32
    pl = {k: np.asarray(v, dtype=np.float32) for k, v in inputs.items()}
    p = pl["p"]
    n = p.shape[0]
    bnd = pl["boundaries"]            # [8, 2, 3]
    lo, hi = bnd[:, 0], bnd[:, 1]

    # exact first-match routing on host (float comparisons are exact) ->
    # bucket points by submap so each core serves one submap table slice.
    # y/z slab extents are identical for every submap, so for in-range
    # inputs only the x comparisons decide; verify that assumption exactly.
    assert (lo[:, 1:] == lo[0, 1:]).all() and (hi[:, 1:] == hi[0, 1:]).all()
    yz_ok = ((p[:, 1:] > lo[0, 1:]) & (p[:, 1:] < hi[0, 1:])).all()
    assert yz_ok, "kernel assumes y/z always in range"
    x = p[:, 0]
    inside = (x[None] > lo[:, None, 0]) & (x[None] < hi[:, None, 0])
    s_star = np.argmax(inside, axis=0).astype(np.int32)
    valid_all = np.any(inside, axis=0)
    assert valid_all.all(), "kernel assumes all points route to a submap"
    npc = nt * PTILE
    counts = np.bincount(s_star, minlength=NCORES)
    assert counts.max() <= npc, f"bucket overflow: {counts} vs {npc}"
    # stable bucket order by submap via argsort on the (small-int) key
    order = np.argsort(s_star, kind="stable")
    splits = np.cumsum(counts)[:-1]
    idx_lists = np.split(order, splits)

    # int8 x-pair tables: row o*32768 + y*128 + k = fused channels of cells
    # (y,2k),(y,2k+1); one global symmetric scale, folded into W1 below.
    plane_keys = ("planes_xy", "c_planes_xy", "planes_xz", "c_planes_xz",
                  "planes_yz", "c_planes_yz")
    from concurrent.futures import ThreadPoolExecutor
    pool = ThreadPoolExecutor(8)
    absmaxes = list(pool.map(lambda k: float(np.abs(pl[k]).max()), plane_keys))
    t_scale = np.float32(max(absmaxes) / 127.0)
    inv_scale = np.float32(1.0) / t_scale
    TA = np.zeros((S, NROWS, 128), np.int8)

    def _fill(o, a, b):
        blk = TA[:, o * NROW_O:(o + 1) * NROW_O].reshape(S, R, R // 2, 2, 64)
        blk[:, :, :, :, :C] = np.clip(
            np.rint(pl[a].reshape(S, R, R // 2, 2, C) * inv_scale), -127, 127)
        blk[:, :, :, :, C:] = np.clip(
            np.rint(pl[b].reshape(S, R, R // 2, 2, C) * inv_scale), -127, 127)

    futs = [pool.submit(_fill, o, a, b)
            for o, (a, b) in enumerate((("planes_xy", "c_planes_xy"),
                                        ("planes_xz", "c_planes_xz"),
                                        ("planes_yz", "c_planes_yz")))]

    w1 = np.zeros((64, 64), f)
    w1[0:32, 0:32] = pl["w0"]
    w1[32:64, 32:64] = pl["cw0"]
    w1 *= t_scale
    w2 = np.zeros((64, 64), f)
    w2[0:32, 0:32] = pl["w1"]
    w2[32:64, 32:64] = pl["cw1"]
    w3 = np.zeros((64, 36), f)
    w3[32:64, 0:3] = pl["cw_out"]
    w3[0:32, 32] = pl["w_out"][:, 0]
    b1 = np.concatenate([pl["b0"], pl["cb0"]]).astype(f)
    b2 = np.concatenate([pl["b1"], pl["cb1"]]).astype(f)
    b3 = np.concatenate([pl["cb_out"], pl["b_out"]]).astype(f)

    # row-index coefficients over (xh_x, xh_y, gy, gz, 1); the per-
    # orientation base o*32768 rides in the constant column
    m3 = np.array([
        [1, 0, 128, 0, 0],
        [1, 0, 0, 128, NROW_O],
        [0, 1, 0, 128, 2 * NROW_O],
    ], f)

    def _core_map(c):
        cst = np.concatenate([
            m3.ravel(), lo[c],
            (np.float32(R - 1) / (hi[c] - lo[c])).astype(f)]).astype(f)
        assert cst.size == NCONST
        wpka = np.zeros((64, 168), f)
        wpka[:, 0:64] = w1
        wpka[:, 64:128] = w2
        wpka[:, 128:164] = w3
        wpka[:, 164] = b1
        wpka[:, 165] = b2
        wpka[0:4, 166] = b3
        wpka[0:NCONST, 167] = cst
        pc = np.full((npc, 3), 0.5, f)
        pc[:len(idx_lists[c])] = p[idx_lists[c]]
        return dict(p_in=np.ascontiguousarray(pc.reshape(nt, 128, KJ, 3)),
                    tabs_all=TA[c], wpk=wpka)

    core_futs = [pool.submit(_core_map, c) for c in range(NCORES)]
    for fu in futs:
        fu.result()
    in_maps = [fu.result() for fu in core_futs]
    pool.shutdown()
    return in_maps, n, idx_lists


def _unscramble(res_list, nt, n, idx_lists):
    out = np.zeros((n, 4), np.float32)

    def _one(c):
        res = res_list[c]
        o = res["out4"].astype(np.float32).reshape(nt, 4, KJ, 128)  # (t,ch,j,p)
        o = o.transpose(0, 3, 2, 1).reshape(nt * PTILE, 4)
        ids = idx_lists[c]
        out[ids] = o[:len(ids)]          # bucket index sets are disjoint

    from concurrent.futures import ThreadPoolExecutor
    with ThreadPoolExecutor(NCORES) as pool:
        list(pool.map(_one, range(NCORES)))
    return out


def run(inputs, nt=NT_FULL, trace=False):
    global LAST_RESULTS
    nc = _get_program(nt)
    in_maps, n, idx_lists = _host_prep(inputs, nt)
    t0 = time.time()
    br = run_bass_kernel_spmd(nc, in_maps, core_ids=list(range(NCORES)),
                              trace=trace)
    print(f"[kernel] run_bass_kernel_spmd took {time.time()-t0:.1f}s "
          f"(exec_time_ns={br.exec_time_ns})", file=sys.stderr)
    LAST_RESULTS = br
    return _unscramble(br.results, nt, n, idx_lists)


def kernel(**inputs):
    trace = bool(int(os.environ.get("KERNEL_TRACE", "0")))
    return run(inputs, nt=NT_FULL, trace=trace)
